# revision 1
# baseline (speedup 1.0000x reference)
"""MAGNN metapath-specific layer (gather + per-edge GRU + edge-softmax +
scatter-sum) on 8 Trainium2 NeuronCores.

Strategy
--------
Host (index-only preprocessing):
  * sort edges by dst; split into 8 contiguous dst-ranges with ~E/8 edges
    each -> every core owns a disjoint output slice (no collectives).
  * pack each core's edges into 128-edge groups such that no dst segment
    crosses a group boundary (pad edges get dst=PAD_ROW and are dropped by
    the scatter bounds check).

Device (per core, identical program, different index data):
  * indirect-DMA gather of metapath node features (3x [128,64] per group)
  * feature-major GRU: h kept transposed [512, e] so no per-step
    transposes; gi+gh accumulated in PSUM; gate biases fused into ACT.
  * attention logits via block-diag attn matmul; softmax without
    max-subtraction (tanh-bounded h keeps logits small); exp computed as
    (1+tanh(a/2))/(1-tanh(a/2)) to stay inside the sigmoid/tanh ACT
    table set; normalization applied after segment aggregation.
  * per-group segment sums via is_equal selection matmul; per-edge rows
    scattered with indirect DMA (duplicates write identical bytes).
"""
import numpy as np
from contextlib import ExitStack

N_CORES = 8
GROUP = 128
PAD_ROW = 1 << 20
H, D, HD, L = 8, 64, 512, 3
LEAKY = 0.01

_RUNNER_CACHE: dict = {}


# ----------------------------------------------------------------- host plan
def _plan(edge_metapath_indices, dst, num_dst):
    E = dst.shape[0]
    order = np.argsort(dst, kind="stable")
    dst_s = dst[order].astype(np.int64)
    idx_s = edge_metapath_indices[order].astype(np.int32)

    seg_starts = np.flatnonzero(np.r_[True, dst_s[1:] != dst_s[:-1]])
    seg_ends = np.r_[seg_starts[1:], E]
    seg_sizes = seg_ends - seg_starts

    cuts = [0]
    for c in range(1, N_CORES):
        target = round(E * c / N_CORES)
        cuts.append(int(np.searchsorted(seg_starts, target, side="left")))
    cuts.append(len(seg_starts))

    cores = []
    prev_hi = 0
    for c in range(N_CORES):
        s0, s1 = cuts[c], cuts[c + 1]
        lo = prev_hi
        hi = num_dst if c == N_CORES - 1 else (
            int(dst_s[seg_starts[s1 - 1]]) + 1 if s1 > s0 else lo)
        prev_hi = hi
        groups = []
        cur, cur_edges = [], 0
        for s in range(s0, s1):
            sz = int(seg_sizes[s])
            assert sz <= GROUP, f"segment larger than a group: {sz}"
            if cur_edges + sz > GROUP:
                groups.append(cur)
                cur, cur_edges = [], 0
            cur.append(s)
            cur_edges += sz
        if cur:
            groups.append(cur)
        cores.append(dict(lo=lo, hi=hi, groups=groups))

    G = max(len(ci["groups"]) for ci in cores)
    G = (G + 3) // 4 * 4
    R = max(ci["hi"] - ci["lo"] for ci in cores)

    per_core = []
    for ci in cores:
        lo = ci["lo"]
        nidx = np.zeros((128, 3 * G), np.int32)       # [edge, 3*g + t]
        drow_i = np.full((128, G), PAD_ROW, np.int32)
        for g, segs in enumerate(ci["groups"]):
            p = 0
            for s in segs:
                a, b = int(seg_starts[s]), int(seg_ends[s])
                n = b - a
                for t in range(L):
                    nidx[p:p + n, 3 * g + t] = idx_s[a:b, t]
                drow_i[p:p + n, g] = dst_s[a:b] - lo
                p += n
        per_core.append(dict(lo=lo, hi=ci["hi"], nidx=nidx, drow_i=drow_i))
    return dict(cores=per_core, G=G, R=R)


def _host_arrays(plan, features, W_ih, W_hh, b_ih, b_hh, attn):
    """Shared (replicated) weight-derived arrays, laid out for the kernel."""
    f32 = np.float32
    w_ihT = np.ascontiguousarray(W_ih.T.astype(f32))              # [64, 1536]
    whhT = W_hh.T.astype(f32)                                     # [512, 1536]
    w_hhT = np.concatenate([whhT[k * 128:(k + 1) * 128] for k in range(4)],
                           axis=1).astype(f32)                    # [128, 6144]
    bsum = (b_ih + b_hh).astype(f32)
    bias = np.zeros((128, 20), f32)
    for c in range(8):
        bias[:, c] = bsum[c * 128:(c + 1) * 128]
    for c4 in range(4):
        bias[:, 8 + c4] = b_ih[1024 + c4 * 128:1024 + (c4 + 1) * 128]
        bias[:, 12 + c4] = b_hh[1024 + c4 * 128:1024 + (c4 + 1) * 128]
        bias[:, 16 + c4] = -bsum[512 + c4 * 128:512 + (c4 + 1) * 128]
    A = np.zeros((HD, H), f32)
    for h in range(H):
        A[h * D:(h + 1) * D, h] = attn[0, h]
    attnA = np.concatenate([A[k * 128:(k + 1) * 128] for k in range(4)],
                           axis=1)                                # [128, 32]
    return dict(w_ihT=w_ihT, w_hhT=np.ascontiguousarray(w_hhT),
                biases=bias, attnA=np.ascontiguousarray(attnA),
                features=np.ascontiguousarray(features.astype(f32)))


# ------------------------------------------------------------ device program
def _build_program(n_nodes, G, R):
    import concourse.bass as bass
    import concourse.tile as tile
    import concourse.mybir as mybir
    from concourse import bacc
    from concourse.masks import make_identity

    f32, i32 = mybir.dt.float32, mybir.dt.int32
    AF = mybir.ActivationFunctionType
    OP = mybir.AluOpType
    NST = G // 4

    nc = bacc.Bacc("TRN2", target_bir_lowering=False, debug=False,
                   num_devices=N_CORES)
    feat = nc.dram_tensor("features", [n_nodes, D], f32, kind="ExternalInput").ap()
    wih_d = nc.dram_tensor("w_ihT", [64, 3 * HD], f32, kind="ExternalInput").ap()
    whh_d = nc.dram_tensor("w_hhT", [128, 4 * 3 * HD], f32, kind="ExternalInput").ap()
    bias_d = nc.dram_tensor("biases", [128, 20], f32, kind="ExternalInput").ap()
    attnA_d = nc.dram_tensor("attnA", [128, 32], f32, kind="ExternalInput").ap()
    idx_d = nc.dram_tensor("nidx", [128, 3 * G], i32, kind="ExternalInput").ap()
    drow_i_d = nc.dram_tensor("drow_i", [128, G], i32, kind="ExternalInput").ap()
    drow_f_d = nc.dram_tensor("drow_f", [128, G], f32, kind="ExternalInput").ap()
    out_d = nc.dram_tensor("out", [R, HD], f32, kind="ExternalOutput").ap()

    with tile.TileContext(nc) as tc, ExitStack() as ctx:
        const = ctx.enter_context(tc.tile_pool(name="const", bufs=1))
        p_mm = ctx.enter_context(tc.tile_pool(name="p_mm", bufs=4, space="PSUM"))
        p_aux = ctx.enter_context(tc.tile_pool(name="p_aux", bufs=4, space="PSUM"))
        xg_pool = ctx.enter_context(tc.tile_pool(name="xg", bufs=8))
        xt_pool = ctx.enter_context(tc.tile_pool(name="xt", bufs=6))
        h_pool = ctx.enter_context(tc.tile_pool(name="h", bufs=4))
        gate_pool = ctx.enter_context(tc.tile_pool(name="gate", bufs=2))
        sm_pool = ctx.enter_context(tc.tile_pool(name="sm", bufs=2))
        out_pool = ctx.enter_context(tc.tile_pool(name="outp", bufs=3))

        ident = const.tile([128, 128], f32)
        make_identity(nc, ident[:])
        wih = const.tile([64, 3 * HD], f32)
        nc.sync.dma_start(out=wih[:], in_=wih_d[:])
        whh = const.tile([128, 12 * HD], f32)
        nc.sync.dma_start(out=whh[:], in_=whh_d[:])
        bias = const.tile([128, 20], f32)
        nc.sync.dma_start(out=bias[:], in_=bias_d[:])
        attnA = const.tile([128, 32], f32)
        nc.sync.dma_start(out=attnA[:], in_=attnA_d[:])
        idx_sb = const.tile([128, 3 * G], i32)
        nc.sync.dma_start(out=idx_sb[:], in_=idx_d[:])
        drow_i_sb = const.tile([128, G], i32)
        nc.sync.dma_start(out=drow_i_sb[:], in_=drow_i_d[:])
        drow_f_sb = const.tile([128, G], f32)
        nc.sync.dma_start(out=drow_f_sb[:], in_=drow_f_d[:])

        bc = nc.gpsimd.to_reg(R - 1)

        def b_ap(col):                       # per-partition bias column
            return bias[:, col:col + 1]

        def hview(t):                        # [128, 512] -> [128, 8, 64]
            return t.rearrange("p (h d) -> p h d", h=H)

        for st in range(NST):
            g0 = 4 * st
            # ---- gather node features for 4 groups ----------------------
            xgs = []
            for g in range(4):
                xg = xg_pool.tile([128, 3 * D], f32, tag="xg")
                for t in range(L):
                    nc.gpsimd.indirect_dma_start(
                        out=xg[:, t * D:(t + 1) * D], out_offset=None,
                        in_=feat[:],
                        in_offset=bass.IndirectOffsetOnAxis(
                            ap=idx_sb[:, 3 * (g0 + g) + t:3 * (g0 + g) + t + 1],
                            axis=0))
                xgs.append(xg)
            # ---- transpose to feature-major xT[t] = [64, 512] -----------
            xts = []
            for t in range(L):
                ps_xt = p_aux.tile([64, 512], f32, tag="aux")
                for g in range(4):
                    nc.tensor.transpose(out=ps_xt[:, g * 128:(g + 1) * 128],
                                        in_=xgs[g][:, t * D:(t + 1) * D],
                                        identity=ident[:])
                xt = xt_pool.tile([64, 512], f32, tag="xt")
                nc.scalar.copy(xt[:], ps_xt[:])
                xts.append(xt)

            # ---- GRU step 1 (h0 = 0) ------------------------------------
            h1 = h_pool.tile([128, 4 * 512], f32, tag="h")
            for c4 in range(4):
                ps_r = p_mm.tile([128, 512], f32, tag="mm")
                nc.tensor.matmul(ps_r[:], lhsT=wih[:, c4 * 128:(c4 + 1) * 128],
                                 rhs=xts[0][:], start=True, stop=True)
                r_t = gate_pool.tile([128, 512], f32, tag="r")
                nc.scalar.activation(r_t[:], ps_r[:], AF.Sigmoid, bias=b_ap(c4))

                ps_z = p_mm.tile([128, 512], f32, tag="mm")
                nc.tensor.matmul(ps_z[:],
                                 lhsT=wih[:, (4 + c4) * 128:(5 + c4) * 128],
                                 rhs=xts[0][:], start=True, stop=True)
                omz_t = gate_pool.tile([128, 512], f32, tag="z")
                nc.scalar.activation(omz_t[:], ps_z[:], AF.Sigmoid,
                                     bias=b_ap(16 + c4), scale=-1.0)

                ps_gin = p_mm.tile([128, 512], f32, tag="mm")
                nc.tensor.matmul(ps_gin[:],
                                 lhsT=wih[:, (8 + c4) * 128:(9 + c4) * 128],
                                 rhs=xts[0][:], start=True, stop=True)
                t1 = gate_pool.tile([128, 512], f32, tag="t1")
                nc.vector.tensor_scalar_mul(t1[:], r_t[:], b_ap(12 + c4))
                t2 = gate_pool.tile([128, 512], f32, tag="t2")
                nc.vector.tensor_add(t2[:], t1[:], ps_gin[:])
                n_t = gate_pool.tile([128, 512], f32, tag="n")
                nc.scalar.activation(n_t[:], t2[:], AF.Tanh, bias=b_ap(8 + c4))
                nc.vector.tensor_mul(h1[:, c4 * 512:(c4 + 1) * 512],
                                     omz_t[:], n_t[:])

            # ---- GRU steps 2 and 3 --------------------------------------
            h_prev = h1
            for s in (1, 2):
                h_new = h_pool.tile([128, 4 * 512], f32, tag="h")
                for c4 in range(4):
                    ps_r = p_mm.tile([128, 512], f32, tag="mm")
                    nc.tensor.matmul(ps_r[:],
                                     lhsT=wih[:, c4 * 128:(c4 + 1) * 128],
                                     rhs=xts[s][:], start=True, stop=False)
                    for k in range(4):
                        nc.tensor.matmul(
                            ps_r[:],
                            lhsT=whh[:, (k * 12 + c4) * 128:(k * 12 + c4 + 1) * 128],
                            rhs=h_prev[:, k * 512:(k + 1) * 512],
                            start=False, stop=(k == 3))
                    r_t = gate_pool.tile([128, 512], f32, tag="r")
                    nc.scalar.activation(r_t[:], ps_r[:], AF.Sigmoid,
                                         bias=b_ap(c4))

                    ps_z = p_mm.tile([128, 512], f32, tag="mm")
                    nc.tensor.matmul(ps_z[:],
                                     lhsT=wih[:, (4 + c4) * 128:(5 + c4) * 128],
                                     rhs=xts[s][:], start=True, stop=False)
                    for k in range(4):
                        nc.tensor.matmul(
                            ps_z[:],
                            lhsT=whh[:, (k * 12 + 4 + c4) * 128:(k * 12 + 5 + c4) * 128],
                            rhs=h_prev[:, k * 512:(k + 1) * 512],
                            start=False, stop=(k == 3))
                    z_t = gate_pool.tile([128, 512], f32, tag="z")
                    nc.scalar.activation(z_t[:], ps_z[:], AF.Sigmoid,
                                         bias=b_ap(4 + c4))

                    ps_gin = p_mm.tile([128, 512], f32, tag="mm")
                    nc.tensor.matmul(ps_gin[:],
                                     lhsT=wih[:, (8 + c4) * 128:(9 + c4) * 128],
                                     rhs=xts[s][:], start=True, stop=True)
                    ps_ghn = p_mm.tile([128, 512], f32, tag="mm")
                    for k in range(4):
                        nc.tensor.matmul(
                            ps_ghn[:],
                            lhsT=whh[:, (k * 12 + 8 + c4) * 128:(k * 12 + 9 + c4) * 128],
                            rhs=h_prev[:, k * 512:(k + 1) * 512],
                            start=(k == 0), stop=(k == 3))
                    ghn = gate_pool.tile([128, 512], f32, tag="ghn")
                    nc.scalar.activation(ghn[:], ps_ghn[:], AF.Identity,
                                         bias=b_ap(12 + c4))
                    t1 = gate_pool.tile([128, 512], f32, tag="t1")
                    nc.vector.tensor_mul(t1[:], r_t[:], ghn[:])
                    t2 = gate_pool.tile([128, 512], f32, tag="t2")
                    nc.vector.tensor_add(t2[:], t1[:], ps_gin[:])
                    n_t = gate_pool.tile([128, 512], f32, tag="n")
                    nc.scalar.activation(n_t[:], t2[:], AF.Tanh,
                                         bias=b_ap(8 + c4))
                    d_t = gate_pool.tile([128, 512], f32, tag="d")
                    nc.vector.tensor_sub(d_t[:],
                                         h_prev[:, c4 * 512:(c4 + 1) * 512],
                                         n_t[:])
                    zd = gate_pool.tile([128, 512], f32, tag="zd")
                    nc.vector.tensor_mul(zd[:], z_t[:], d_t[:])
                    nc.vector.tensor_add(h_new[:, c4 * 512:(c4 + 1) * 512],
                                         n_t[:], zd[:])
                h_prev = h_new
            h3 = h_prev

            # ---- attention logits + p = exp(leakyrelu(a)) ---------------
            ps_a = p_aux.tile([128, 32], f32, tag="aux")
            for g in range(4):
                for k in range(4):
                    nc.tensor.matmul(
                        ps_a[:, g * 8:(g + 1) * 8],
                        lhsT=h3[:, k * 512 + g * 128:k * 512 + (g + 1) * 128],
                        rhs=attnA[:, k * 8:(k + 1) * 8],
                        start=(k == 0), stop=(k == 3))
            ta = sm_pool.tile([128, 32], f32, tag="ta")
            nc.vector.tensor_scalar_mul(ta[:], ps_a[:], LEAKY)
            al = sm_pool.tile([128, 32], f32, tag="al")
            nc.vector.tensor_tensor(out=al[:], in0=ps_a[:], in1=ta[:], op=OP.max)
            th = sm_pool.tile([128, 32], f32, tag="th")
            nc.scalar.activation(th[:], al[:], AF.Tanh, scale=0.5)
            nm = sm_pool.tile([128, 32], f32, tag="nm")
            nc.vector.tensor_scalar_add(nm[:], th[:], 1.0)
            dn = sm_pool.tile([128, 32], f32, tag="dn")
            nc.vector.tensor_scalar(dn[:], th[:], -1.0, 1.0, OP.mult, OP.add)
            rd = sm_pool.tile([128, 32], f32, tag="rd")
            nc.vector.reciprocal(rd[:], dn[:])
            p_st = sm_pool.tile([128, 32], f32, tag="p")
            nc.vector.tensor_mul(p_st[:], nm[:], rd[:])

            # ---- per-group segment softmax-sum + scatter ----------------
            for g in range(4):
                gabs = g0 + g
                ps_h3g = p_aux.tile([128, 512], f32, tag="aux")
                for k in range(4):
                    nc.tensor.transpose(
                        out=ps_h3g[:, k * 128:(k + 1) * 128],
                        in_=h3[:, k * 512 + g * 128:k * 512 + (g + 1) * 128],
                        identity=ident[:])
                wg = out_pool.tile([128, 512], f32, tag="wg")
                nc.vector.tensor_tensor(
                    out=hview(wg[:]), in0=hview(ps_h3g[:]),
                    in1=p_st[:, g * 8:(g + 1) * 8, None].to_broadcast([128, H, D]),
                    op=OP.mult)
                ps_d1t = p_aux.tile([128, 128], f32, tag="aux")
                nc.tensor.transpose(
                    out=ps_d1t[:],
                    in_=drow_f_sb[:, gabs:gabs + 1].to_broadcast([128, 128]),
                    identity=ident[:])
                S_t = out_pool.tile([128, 128], f32, tag="S")
                nc.vector.tensor_tensor(
                    out=S_t[:],
                    in0=drow_f_sb[:, gabs:gabs + 1].to_broadcast([128, 128]),
                    in1=ps_d1t[:], op=OP.is_equal)
                ps_z = p_aux.tile([128, 512], f32, tag="aux")
                nc.tensor.matmul(ps_z[:], lhsT=S_t[:], rhs=wg[:],
                                 start=True, stop=True)
                ps_s = p_aux.tile([128, 8], f32, tag="aux")
                nc.tensor.matmul(ps_s[:], lhsT=S_t[:],
                                 rhs=p_st[:, g * 8:(g + 1) * 8],
                                 start=True, stop=True)
                rec = sm_pool.tile([128, 8], f32, tag="rec")
                nc.vector.reciprocal(rec[:], ps_s[:])
                zo = out_pool.tile([128, 512], f32, tag="zo")
                nc.vector.tensor_tensor(
                    out=hview(zo[:]), in0=hview(ps_z[:]),
                    in1=rec[:, :, None].to_broadcast([128, H, D]),
                    op=OP.mult)
                nc.gpsimd.indirect_dma_start(
                    out=out_d[:],
                    out_offset=bass.IndirectOffsetOnAxis(
                        ap=drow_i_sb[:, gabs:gabs + 1], axis=0),
                    in_=zo[:], in_offset=None,
                    bounds_check=bc, oob_is_err=False)

    nc.compile()
    return nc


# ------------------------------------------------------------------- driver
def _get_program(n_nodes, G, R):
    key = (n_nodes, G, R)
    if key not in _RUNNER_CACHE:
        _RUNNER_CACHE[key] = _build_program(n_nodes, G, R)
    return _RUNNER_CACHE[key]


def run_on_device(plan, shared, n_nodes, trace=False):
    from concourse.bass_utils import run_bass_kernel_spmd
    nc = _get_program(n_nodes, plan["G"], plan["R"])
    in_maps = []
    for ci in plan["cores"]:
        in_maps.append({
            "features": shared["features"], "w_ihT": shared["w_ihT"],
            "w_hhT": shared["w_hhT"], "biases": shared["biases"],
            "attnA": shared["attnA"], "nidx": ci["nidx"],
            "drow_i": ci["drow_i"],
            "drow_f": ci["drow_i"].astype(np.float32),
        })
    res = run_bass_kernel_spmd(nc, in_maps, list(range(N_CORES)), trace=trace)
    return res


def kernel(features, W_ih, W_hh, b_ih, b_hh, attn,
           edge_metapath_indices, dst, num_dst):
    num_dst = int(num_dst)
    plan = _plan(np.asarray(edge_metapath_indices), np.asarray(dst), num_dst)
    shared = _host_arrays(plan, np.asarray(features), np.asarray(W_ih),
                          np.asarray(W_hh), np.asarray(b_ih),
                          np.asarray(b_hh), np.asarray(attn))
    res = run_on_device(plan, shared, features.shape[0])
    out = np.zeros((num_dst, HD), np.float32)
    for c, ci in enumerate(plan["cores"]):
        lo, hi = ci["lo"], ci["hi"]
        out[lo:hi] = res.results[c]["out"][:hi - lo]
    return out.reshape(num_dst, H, D)
